# revision 1
# baseline (speedup 1.0000x reference)
"""Causal multi-head attention on 8 TRN2 NeuronCores.

Problem: query/key/value [2048, 4, 16, 128] f32, causal mask, softmax(QK^T/sqrt(128)) @ V,
output [2048, 4, 2048] f32.

Sharding: the 4*16 = 64 (batch, head) pairs split as 8 pairs per core; each core
computes fully local attention for its pairs (no collectives).

Host-side prep (outside HW exec): cast to bf16 and pre-transpose Q, K to
[pair, hn=128, sq=2048] so the device kernel loads contraction-major tiles
directly. V stays [pair, sq, hn].

Device kernel per pair:
  - S^T tile [k=128, q<=512] = matmul(lhsT=K^T k-slice, rhs=Q^T q-group) in PSUM
    (only causal k-tiles; diagonal tiles at exact reduced width)
  - P^T = exp(scale * S^T) via ScalarE, writing bf16 to SBUF; causally-invalid
    entries zeroed by one gpsimd affine_select per diagonal chunk
  - out [q=128, 129] accumulates matmul(lhsT=P^T block, rhs=[V k-tile | ones]) over
    k-tiles; column 128 is the softmax denominator (ones column trick)
  - DVE reciprocal + per-partition multiply normalizes; DMA out per 512-row group
"""

import sys
import types

import numpy as np
import ml_dtypes

SQ, B, NP, HN = 2048, 4, 16, 128
N_CORES = 8
PAIRS = B * NP
PAIRS_PER_CORE = PAIRS // N_CORES
SCALE = 1.0 / np.sqrt(HN).astype(np.float32)
N_QT = SQ // 128          # 16 q-tiles of 128
N_KT = SQ // 128          # 16 k-tiles of 128
N_G = SQ // 512           # 4 q-groups of 512
CHUNK = 2                 # k-tiles per PSUM chunk ([128, CHUNK*512] f32 = CHUNK banks)


def _ensure_axon_hooks_stub():
    """bass_utils imports antenv.axon_hooks when tracing is requested; this
    container's antenv lacks it.  Install a stub that disables tracing so a
    stray BASS_TRACE env var can't crash the run.  A real hook installed
    earlier (e.g. by test.py) is left untouched."""
    if "antenv.axon_hooks" in sys.modules:
        return
    try:
        import antenv.axon_hooks  # noqa: F401
    except ImportError:
        mod = types.ModuleType("antenv.axon_hooks")
        mod.get_axon_ntff_profile_hook = lambda: None
        mod.set_axon_ntff_profile_hook = lambda hook: None
        sys.modules["antenv.axon_hooks"] = mod


_NC_CACHE = None


def _build():
    import concourse.bacc as bacc
    import concourse.mybir as mybir
    from concourse.tile import TileContext

    f32 = mybir.dt.float32
    bf16 = mybir.dt.bfloat16

    nc = bacc.Bacc("TRN2", target_bir_lowering=False, debug=False,
                   num_devices=N_CORES)
    qt_d = nc.declare_dram_parameter("qt", [PAIRS_PER_CORE, HN, SQ], bf16,
                                     isOutput=False)
    kt_d = nc.declare_dram_parameter("kt", [PAIRS_PER_CORE, HN, SQ], bf16,
                                     isOutput=False)
    v_d = nc.declare_dram_parameter("v", [PAIRS_PER_CORE, SQ, HN], bf16,
                                    isOutput=False)
    out_d = nc.declare_dram_parameter("out", [PAIRS_PER_CORE, SQ, HN], f32,
                                      isOutput=True)

    with TileContext(nc) as tc:
        with (
            tc.tile_pool(name="qk", bufs=2) as qk_pool,
            tc.tile_pool(name="vp", bufs=2) as v_pool,
            tc.tile_pool(name="pt", bufs=12) as p_pool,
            tc.tile_pool(name="og", bufs=3) as og_pool,
            tc.tile_pool(name="sm", bufs=4) as sm_pool,
            tc.tile_pool(name="sps", bufs=3, space="PSUM") as s_pool,
            tc.tile_pool(name="ops", bufs=2, space="PSUM") as o_pool,
        ):
            for p in range(PAIRS_PER_CORE):
                qt_sb = qk_pool.tile([HN, SQ], bf16, tag="qt")
                kt_sb = qk_pool.tile([HN, SQ], bf16, tag="kt")
                nc.sync.dma_start(out=qt_sb, in_=qt_d[p])
                nc.sync.dma_start(out=kt_sb, in_=kt_d[p])
                v_sb = v_pool.tile([128, N_KT, 130], bf16, tag="v")
                nc.sync.dma_start(
                    out=v_sb[:, :, 0:HN],
                    in_=v_d[p].rearrange("(j q) h -> q j h", q=128),
                )
                nc.gpsimd.memset(v_sb[:, :, HN:HN + 1], 1.0)

                for g in range(N_G):
                    n_k = 4 * g + 4  # causal k-tiles for this q-group
                    pt_tiles = []
                    for c0 in range(0, n_k, CHUNK):
                        cw = min(CHUNK, n_k - c0)
                        ps = s_pool.tile([128, CHUNK * 512], f32, tag="s")
                        for ci in range(cw):
                            j = c0 + ci
                            r = j - 4 * g  # diagonal sub-tile index (>=0 on diag)
                            lo = 128 * r if r > 0 else 0
                            nc.tensor.matmul(
                                ps[:, ci * 512 + lo:(ci + 1) * 512],
                                lhsT=kt_sb[:, j * 128:(j + 1) * 128],
                                rhs=qt_sb[:, g * 512 + lo:(g + 1) * 512],
                                start=True, stop=True,
                            )
                        pt = p_pool.tile([128, CHUNK * 512], bf16, tag="p")
                        nc.scalar.activation(
                            pt[:, :cw * 512], ps[:, :cw * 512],
                            mybir.ActivationFunctionType.Exp, scale=float(SCALE),
                        )
                        r0 = c0 - 4 * g
                        if c0 + cw > 4 * g:
                            # chunk holds diagonal tiles: zero entries with k > q
                            # keep where  -p + c - 128*(r0 + n) >= 0
                            sel = pt[:, :cw * 512].rearrange(
                                "q (n c) -> q n c", c=512)
                            nc.gpsimd.affine_select(
                                out=sel, in_=sel,
                                compare_op=mybir.AluOpType.is_ge,
                                fill=0.0,
                                base=-128 * r0,
                                pattern=[[-128, cw], [1, 512]],
                                channel_multiplier=-1,
                            )
                        pt_tiles.append(pt)

                    out_sb = og_pool.tile([128, 4, HN], f32, tag="og")
                    for u in range(4):
                        t = 4 * g + u
                        po = o_pool.tile([128, 130], f32, tag="o")
                        for j in range(t + 1):
                            cidx, ci = divmod(j, CHUNK)
                            pt = pt_tiles[cidx]
                            nc.tensor.matmul(
                                po[:, 0:HN + 1],
                                lhsT=pt[:, ci * 512 + u * 128:
                                        ci * 512 + u * 128 + 128],
                                rhs=v_sb[:, j, 0:HN + 1],
                                start=(j == 0), stop=(j == t),
                            )
                        rec = sm_pool.tile([128, 1], f32, tag="rec")
                        nc.vector.reciprocal(rec, po[:, HN:HN + 1])
                        nc.vector.tensor_scalar_mul(
                            out_sb[:, u, :], po[:, 0:HN], rec)
                    nc.sync.dma_start(
                        out=out_d[p, g * 512:(g + 1) * 512, :].rearrange(
                            "(t q) h -> q t h", q=128),
                        in_=out_sb,
                    )
    nc.finalize()
    return nc


def _get_nc():
    global _NC_CACHE
    if _NC_CACHE is None:
        _NC_CACHE = _build()
    return _NC_CACHE


def _run(in_maps, trace=False, tmpdir=None):
    _ensure_axon_hooks_stub()
    from concourse.bass_utils import run_bass_kernel_spmd

    nc = _get_nc()
    return run_bass_kernel_spmd(nc, in_maps, core_ids=list(range(N_CORES)),
                                trace=trace, tmpdir=tmpdir)


def _make_in_maps(query, key, value):
    bf16 = ml_dtypes.bfloat16
    q = np.asarray(query, dtype=np.float32)
    k = np.asarray(key, dtype=np.float32)
    v = np.asarray(value, dtype=np.float32)
    # [sq, b, np, hn] -> [pair, hn, sq] for q/k ; [pair, sq, hn] for v
    qt = np.ascontiguousarray(q.transpose(1, 2, 3, 0).reshape(PAIRS, HN, SQ)).astype(bf16)
    kt = np.ascontiguousarray(k.transpose(1, 2, 3, 0).reshape(PAIRS, HN, SQ)).astype(bf16)
    vn = np.ascontiguousarray(v.transpose(1, 2, 0, 3).reshape(PAIRS, SQ, HN)).astype(bf16)
    in_maps = []
    for c in range(N_CORES):
        sl = slice(c * PAIRS_PER_CORE, (c + 1) * PAIRS_PER_CORE)
        in_maps.append({
            "qt": np.ascontiguousarray(qt[sl]),
            "kt": np.ascontiguousarray(kt[sl]),
            "v": np.ascontiguousarray(vn[sl]),
        })
    return in_maps


def _gather_out(results):
    outs = [np.asarray(results[c]["out"], dtype=np.float32)
            for c in range(N_CORES)]
    out = np.concatenate(outs, axis=0).reshape(B, NP, SQ, HN)
    return np.ascontiguousarray(
        out.transpose(2, 0, 1, 3).reshape(SQ, B, NP * HN))


def kernel(query, key, value, attention_mask=None, **_unused):
    """Full-input attention: shards over 8 NeuronCores internally.

    attention_mask is the static causal mask from the problem spec; causality
    is hardcoded in the device kernel.
    """
    in_maps = _make_in_maps(query, key, value)
    res = _run(in_maps, trace=False)
    return _gather_out(res.results)
